# revision 2
# baseline (speedup 1.0000x reference)
"""Trainium2 Bass kernel for nn_MeshNN (piecewise-linear hat-basis interpolation).

Math: with uniform node grid c_k = fl(k*h) + c0 (bit-exact linspace
reconstruction) and clamped peaks X_i = max(min(c_i, TP*c_{i+1}), TM*c_{i-1}),
the model output for x in cell j = floor((x-c0)/h) is

    u(x) = [j>=1]*min(rise_j, fall_j) + [j<=NP-3]*min(rise_{j+1}, fall_{j+1})

    rise_i = (x - c_{i-1}) / (X_i - c_{i-1})
    fall_i = (c_{i+1} - x) / (c_{i+1} - X_i)

valid when w_uu == 1, w_dd == 0 (checked on host at call time). This needs no
table lookups: ~30 elementwise ops/point, spread across DVE/ACT/GPSIMD.

Data-parallel over 8 NeuronCores: x is split into 8 shards of 62500 points,
each padded to 128x489 and processed independently; outputs are concatenated.
"""

import os
import numpy as np

N_PTS = 500_000
N_CORES = 8
PER_CORE = N_PTS // N_CORES          # 62500
P = 128
COLS = 489                            # 128*489 = 62592 >= 62500
PAD = P * COLS

_CACHE: dict = {}
LAST_RESULTS = None  # BassKernelResults of the most recent run (for profiling)


def _build_module(h: float, c0: float, np_nodes: int):
    """Build + compile the single-core Bass program (SPMD across 8 cores)."""
    import concourse.bass as bass  # noqa: F401
    import concourse.tile as tile
    from concourse import bacc, mybir

    F32 = mybir.dt.float32
    I32 = mybir.dt.int32
    OP = mybir.AluOpType

    f = np.float32
    TP = float(f(1.0 - 1.0 / 150.0))
    TM = float(f(1.0 + 1.0 / 150.0))
    hf = float(f(h))
    inv_h = float(f(1.0) / f(h))
    c0f = float(f(c0))
    jmax = float(f(np_nodes - 2))     # last cell index (256)

    nc = bacc.Bacc(
        "TRN2", target_bir_lowering=False, debug=False, num_devices=N_CORES
    )
    x_d = nc.dram_tensor("x", [P, COLS], F32, kind="ExternalInput")
    o_d = nc.dram_tensor("out", [P, COLS], F32, kind="ExternalOutput")

    from contextlib import ExitStack

    with tile.TileContext(nc) as tc, ExitStack() as ctx:
        pool = ctx.enter_context(tc.tile_pool(name="work", bufs=1))

        def T(tag, dt=F32):
            return pool.tile([P, COLS], dt, name=tag, tag=tag)

        x = T("x")
        nc.sync.dma_start(x[:], x_d.ap())

        # t = (x - c0) * inv_h   (ACT; c0 folded into bias)
        t = T("t")
        nc.scalar.activation(
            t[:], x[:], mybir.ActivationFunctionType.Copy,
            bias=float(f(-c0f) * f(inv_h)), scale=inv_h,
        )

        # jf = clamp(floor(t), 0, jmax)  -- cast-fixup works for trunc or rne
        ji = T("ji", I32)
        nc.vector.tensor_copy(ji[:], t[:])
        jf0 = T("jf0")
        nc.vector.tensor_copy(jf0[:], ji[:])
        gt = T("gt")
        nc.vector.tensor_tensor(gt[:], jf0[:], t[:], op=OP.is_gt)
        jf1 = T("jf1")
        nc.vector.tensor_sub(jf1[:], jf0[:], gt[:])
        jf = T("jf")
        nc.vector.tensor_scalar(jf[:], jf1[:], jmax, 0.0, OP.min, OP.max)

        # node values c_{j-1}, c_j, c_{j+1}, c_{j+2}: fl((jf+k)*h) + c0
        # (first add is exact: jf+k is a small integer; second op matches
        #  linspace's fl(k*h) rounding bit-for-bit when c0 == 0)
        def node(tag, k):
            c = T(tag)
            if c0f == 0.0:
                nc.gpsimd.tensor_scalar(c[:], jf[:], float(k), hf, OP.add, OP.mult)
            else:
                tmp = T(tag + "_i")
                nc.gpsimd.tensor_scalar(tmp[:], jf[:], float(k), hf, OP.add, OP.mult)
                nc.gpsimd.tensor_scalar(c[:], tmp[:], c0f, None, OP.add)
            return c

        cjm1 = node("cjm1", -1.0)
        cj = node("cj", 0.0)
        cjp1 = node("cjp1", 1.0)
        cjp2 = node("cjp2", 2.0)

        # clamped peaks
        q1 = T("q1")
        nc.vector.scalar_tensor_tensor(q1[:], cjp1[:], TP, cj[:], OP.mult, OP.min)
        Xj = T("Xj")
        nc.vector.scalar_tensor_tensor(Xj[:], cjm1[:], TM, q1[:], OP.mult, OP.max)
        q2 = T("q2")
        nc.vector.scalar_tensor_tensor(q2[:], cjp2[:], TP, cjp1[:], OP.mult, OP.min)
        Xjp1 = T("Xjp1")
        nc.vector.scalar_tensor_tensor(Xjp1[:], cj[:], TM, q2[:], OP.mult, OP.max)

        # denominators and numerators
        d2 = T("d2")
        nc.gpsimd.tensor_sub(d2[:], Xj[:], cjm1[:])
        d3 = T("d3")
        nc.gpsimd.tensor_sub(d3[:], cjp1[:], Xj[:])
        d4 = T("d4")
        nc.gpsimd.tensor_sub(d4[:], Xjp1[:], cj[:])
        d5 = T("d5")
        nc.gpsimd.tensor_sub(d5[:], cjp2[:], Xjp1[:])

        n2 = T("n2")
        nc.gpsimd.tensor_sub(n2[:], x[:], cjm1[:])
        n3 = T("n3")
        nc.gpsimd.tensor_sub(n3[:], cjp1[:], x[:])
        n4 = T("n4")
        nc.gpsimd.tensor_sub(n4[:], x[:], cj[:])
        n5 = T("n5")
        nc.gpsimd.tensor_sub(n5[:], cjp2[:], x[:])

        # reciprocals (~51 ulp; error is ~3e-6 absolute on u)
        r2 = T("r2")
        nc.vector.reciprocal_approx_fast(r2[:], d2[:])
        r3 = T("r3")
        nc.vector.reciprocal_approx_fast(r3[:], d3[:])
        r4 = T("r4")
        nc.vector.reciprocal_approx_fast(r4[:], d4[:])
        r5 = T("r5")
        nc.vector.reciprocal_approx_fast(r5[:], d5[:])

        rj = T("rj")
        nc.vector.tensor_mul(rj[:], n2[:], r2[:])
        fj = T("fj")
        nc.vector.tensor_mul(fj[:], n3[:], r3[:])
        rj1 = T("rj1")
        nc.vector.tensor_mul(rj1[:], n4[:], r4[:])
        fj1 = T("fj1")
        nc.vector.tensor_mul(fj1[:], n5[:], r5[:])

        A = T("A")
        nc.vector.tensor_tensor(A[:], rj[:], fj[:], op=OP.min)
        B = T("B")
        nc.vector.tensor_tensor(B[:], rj1[:], fj1[:], op=OP.min)

        # u = (jf>=1)*A + (jf<=jmax-1)*B
        Am = T("Am")
        nc.vector.scalar_tensor_tensor(Am[:], jf[:], 1.0, A[:], OP.is_ge, OP.mult)
        Bm = T("Bm")
        nc.vector.scalar_tensor_tensor(
            Bm[:], jf[:], float(f(jmax) - f(1.0)), B[:], OP.is_le, OP.mult
        )
        u = T("u")
        nc.vector.tensor_add(u[:], Am[:], Bm[:])

        nc.sync.dma_start(o_d.ap(), u[:])

    nc.compile()
    return nc


def _grid_params(coordinates: np.ndarray):
    c = np.asarray(coordinates, dtype=np.float32)
    n = c.shape[0]
    c0 = float(c[0])
    # pick the h whose fl((j)*h)+c0 reconstruction matches bit-for-bit
    cands = [
        np.float32((float(c[-1]) - float(c[0])) / (n - 1)),
        np.float32(c[1]) - np.float32(c[0]),
    ]
    best, best_bad = None, None
    j = np.arange(n, dtype=np.float32)
    for hc in cands:
        recon = (j * hc + np.float32(c0)).astype(np.float32)
        bad = int(np.sum(recon.view(np.int32) != c.view(np.int32)))
        if best_bad is None or bad < best_bad:
            best, best_bad = hc, bad
    assert best is not None
    if best_bad:
        # still fine (error ~1e-4 worst case at peaks), but flag it
        print(f"kernel.py: warning: grid reconstruction mismatches {best_bad}/{n} nodes")
    return float(best), c0, n


def kernel(x, coordinates, w_uu, w_dd):
    global LAST_RESULTS
    from concourse.bass_utils import run_bass_kernel_spmd

    x = np.asarray(x, dtype=np.float32)
    n = x.shape[0]
    assert x.shape == (n, 1) and n == N_PTS, x.shape
    assert np.all(np.asarray(w_uu, np.float32) == 1.0), "kernel assumes w_uu == 1"
    assert np.all(np.asarray(w_dd, np.float32) == 0.0), "kernel assumes w_dd == 0"

    h, c0, np_nodes = _grid_params(coordinates)
    key = (h, c0, np_nodes)
    if key not in _CACHE:
        _CACHE[key] = _build_module(h, c0, np_nodes)
    nc = _CACHE[key]

    flat = x.reshape(-1)
    in_maps = []
    for i in range(N_CORES):
        shard = flat[i * PER_CORE : (i + 1) * PER_CORE]
        buf = np.full(PAD, 5.0, dtype=np.float32)
        buf[:PER_CORE] = shard
        in_maps.append({"x": buf.reshape(P, COLS)})

    res = run_bass_kernel_spmd(
        nc, in_maps, core_ids=list(range(N_CORES)),
        trace=bool(int(os.environ.get("MESH_TRACE", "0"))),
    )
    LAST_RESULTS = res

    out = np.empty(N_PTS, dtype=np.float32)
    for i in range(N_CORES):
        out[i * PER_CORE : (i + 1) * PER_CORE] = (
            res.results[i]["out"].reshape(-1)[:PER_CORE]
        )
    return out.reshape(N_PTS, 1)


# revision 3
# speedup vs baseline: 1.4687x; 1.4687x over previous
"""Trainium2 Bass kernel for nn_MeshNN (piecewise-linear hat-basis interpolation).

Math: with uniform node grid c_k = fl(k*h) + c0 (bit-exact linspace
reconstruction) and clamped peaks X_i = max(min(c_i, TP*c_{i+1}), TM*c_{i-1}),
the model output for x in cell j = floor((x-c0)/h) is

    u(x) = [j>=1]*min(rise_j, fall_j) + [j<=NP-3]*min(rise_{j+1}, fall_{j+1})

    rise_i = (x - c_{i-1}) / (X_i - c_{i-1})
    fall_i = (c_{i+1} - x) / (c_{i+1} - X_i)

valid when w_uu == 1, w_dd == 0 (checked on host at call time). This needs no
table lookups: ~30 elementwise ops/point, spread across DVE/ACT/GPSIMD.

Data-parallel over 8 NeuronCores: x is split into 8 shards of 62500 points,
each padded to 128x489 and processed independently; outputs are concatenated.
"""

import os
import numpy as np

N_PTS = 500_000
N_CORES = 8
PER_CORE = N_PTS // N_CORES          # 62500
P = 128
COLS = 489                            # 128*489 = 62592 >= 62500
PAD = P * COLS

_CACHE: dict = {}
LAST_RESULTS = None  # BassKernelResults of the most recent run (for profiling)


def _build_module(h: float, c0: float, np_nodes: int):
    """Build + compile the single-core Bass program (SPMD across 8 cores)."""
    import concourse.bass as bass  # noqa: F401
    import concourse.tile as tile
    from concourse import bacc, mybir

    F32 = mybir.dt.float32
    I32 = mybir.dt.int32
    OP = mybir.AluOpType

    f = np.float32
    TP = float(f(1.0 - 1.0 / 150.0))
    TM = float(f(1.0 + 1.0 / 150.0))
    hf = float(f(h))
    inv_h = float(f(1.0) / f(h))
    c0f = float(f(c0))
    jmax = float(f(np_nodes - 2))     # last cell index (256)

    nc = bacc.Bacc(
        "TRN2", target_bir_lowering=False, debug=False, num_devices=N_CORES
    )
    x_d = nc.dram_tensor("x", [P, COLS], F32, kind="ExternalInput")
    o_d = nc.dram_tensor("out", [P, COLS], F32, kind="ExternalOutput")

    from contextlib import ExitStack

    with tile.TileContext(nc) as tc, ExitStack() as ctx:
        pool = ctx.enter_context(tc.tile_pool(name="work", bufs=1))

        def T(tag, dt=F32):
            return pool.tile([P, COLS], dt, name=tag, tag=tag)

        x = T("x")
        nc.sync.dma_start(x[:], x_d.ap())

        # t = (x - c0) * inv_h   (ACT; c0 folded into bias)
        t = T("t")
        nc.scalar.activation(
            t[:], x[:], mybir.ActivationFunctionType.Copy,
            bias=float(f(-c0f) * f(inv_h)), scale=inv_h,
        )

        # jf = clamp(floor(t), 0, jmax)  -- cast-fixup works for trunc or rne
        ji = T("ji", I32)
        nc.vector.tensor_copy(ji[:], t[:])
        jf0 = T("jf0")
        nc.vector.tensor_copy(jf0[:], ji[:])
        gt = T("gt")
        nc.vector.tensor_tensor(gt[:], jf0[:], t[:], op=OP.is_gt)
        jf1 = T("jf1")
        nc.vector.tensor_sub(jf1[:], jf0[:], gt[:])
        jf = T("jf")
        nc.vector.tensor_scalar(jf[:], jf1[:], jmax, 0.0, OP.min, OP.max)

        # node values c_{j-1}, c_j, c_{j+1}, c_{j+2}: fl((jf+k)*h) + c0
        # (first add is exact: jf+k is a small integer; second op matches
        #  linspace's fl(k*h) rounding bit-for-bit when c0 == 0)
        def node(tag, k):
            c = T(tag)
            if c0f == 0.0:
                nc.gpsimd.tensor_scalar(c[:], jf[:], float(k), hf, OP.add, OP.mult)
            else:
                tmp = T(tag + "_i")
                nc.gpsimd.tensor_scalar(tmp[:], jf[:], float(k), hf, OP.add, OP.mult)
                nc.gpsimd.tensor_scalar(c[:], tmp[:], c0f, None, OP.add)
            return c

        cjm1 = node("cjm1", -1.0)
        cj = node("cj", 0.0)
        cjp1 = node("cjp1", 1.0)
        cjp2 = node("cjp2", 2.0)

        # clamped peaks
        q1 = T("q1")
        nc.vector.scalar_tensor_tensor(q1[:], cjp1[:], TP, cj[:], OP.mult, OP.min)
        Xj = T("Xj")
        nc.vector.scalar_tensor_tensor(Xj[:], cjm1[:], TM, q1[:], OP.mult, OP.max)
        q2 = T("q2")
        nc.vector.scalar_tensor_tensor(q2[:], cjp2[:], TP, cjp1[:], OP.mult, OP.min)
        Xjp1 = T("Xjp1")
        nc.vector.scalar_tensor_tensor(Xjp1[:], cj[:], TM, q2[:], OP.mult, OP.max)

        # denominators and numerators
        d2 = T("d2")
        nc.gpsimd.tensor_sub(d2[:], Xj[:], cjm1[:])
        d3 = T("d3")
        nc.gpsimd.tensor_sub(d3[:], cjp1[:], Xj[:])
        d4 = T("d4")
        nc.gpsimd.tensor_sub(d4[:], Xjp1[:], cj[:])
        d5 = T("d5")
        nc.gpsimd.tensor_sub(d5[:], cjp2[:], Xjp1[:])

        n2 = T("n2")
        nc.gpsimd.tensor_sub(n2[:], x[:], cjm1[:])
        n3 = T("n3")
        nc.gpsimd.tensor_sub(n3[:], cjp1[:], x[:])
        n4 = T("n4")
        nc.gpsimd.tensor_sub(n4[:], x[:], cj[:])
        n5 = T("n5")
        nc.gpsimd.tensor_sub(n5[:], cjp2[:], x[:])

        # reciprocals (~51 ulp; error is ~3e-6 absolute on u)
        r2 = T("r2")
        nc.vector.reciprocal_approx_fast(r2[:], d2[:])
        r3 = T("r3")
        nc.vector.reciprocal_approx_fast(r3[:], d3[:])
        r4 = T("r4")
        nc.vector.reciprocal_approx_fast(r4[:], d4[:])
        r5 = T("r5")
        nc.vector.reciprocal_approx_fast(r5[:], d5[:])

        rj = T("rj")
        nc.vector.tensor_mul(rj[:], n2[:], r2[:])
        fj = T("fj")
        nc.vector.tensor_mul(fj[:], n3[:], r3[:])
        rj1 = T("rj1")
        nc.vector.tensor_mul(rj1[:], n4[:], r4[:])
        fj1 = T("fj1")
        nc.vector.tensor_mul(fj1[:], n5[:], r5[:])

        A = T("A")
        nc.vector.tensor_tensor(A[:], rj[:], fj[:], op=OP.min)
        B = T("B")
        nc.vector.tensor_tensor(B[:], rj1[:], fj1[:], op=OP.min)

        # u = (jf>=1)*A + (jf<=jmax-1)*B
        Am = T("Am")
        nc.vector.scalar_tensor_tensor(Am[:], jf[:], 1.0, A[:], OP.is_ge, OP.mult)
        Bm = T("Bm")
        nc.vector.scalar_tensor_tensor(
            Bm[:], jf[:], float(f(jmax) - f(1.0)), B[:], OP.is_le, OP.mult
        )
        u = T("u")
        nc.vector.tensor_add(u[:], Am[:], Bm[:])

        nc.sync.dma_start(o_d.ap(), u[:])

    nc.compile()
    return nc


def _grid_params(coordinates: np.ndarray):
    c = np.asarray(coordinates, dtype=np.float32)
    n = c.shape[0]
    c0 = float(c[0])
    # pick the h whose fl((j)*h)+c0 reconstruction matches bit-for-bit
    cands = [
        np.float32((float(c[-1]) - float(c[0])) / (n - 1)),
        np.float32(c[1]) - np.float32(c[0]),
    ]
    best, best_bad = None, None
    j = np.arange(n, dtype=np.float32)
    for hc in cands:
        recon = (j * hc + np.float32(c0)).astype(np.float32)
        bad = int(np.sum(recon.view(np.int32) != c.view(np.int32)))
        if best_bad is None or bad < best_bad:
            best, best_bad = hc, bad
    assert best is not None
    if best_bad:
        # still fine (error ~1e-4 worst case at peaks), but flag it
        print(f"kernel.py: warning: grid reconstruction mismatches {best_bad}/{n} nodes")
    return float(best), c0, n


def kernel(x, coordinates, w_uu, w_dd):
    global LAST_RESULTS
    from concourse.bass_utils import run_bass_kernel_spmd

    x = np.asarray(x, dtype=np.float32)
    n = x.shape[0]
    assert x.shape == (n, 1) and n == N_PTS, x.shape
    assert np.all(np.asarray(w_uu, np.float32) == 1.0), "kernel assumes w_uu == 1"
    assert np.all(np.asarray(w_dd, np.float32) == 0.0), "kernel assumes w_dd == 0"

    h, c0, np_nodes = _grid_params(coordinates)
    ver = os.environ.get("MESH_KERNEL_VER", "v6")
    key = (h, c0, np_nodes, ver)
    if key not in _CACHE:
        if ver == "v6":
            from kernel_v6 import build_v6
            _CACHE[key] = build_v6(h, c0, np_nodes)
        else:
            _CACHE[key] = _build_module(h, c0, np_nodes)
    nc = _CACHE[key]

    flat = x.reshape(-1)
    in_maps = []
    for i in range(N_CORES):
        shard = flat[i * PER_CORE : (i + 1) * PER_CORE]
        buf = np.full(PAD, 5.0, dtype=np.float32)
        buf[:PER_CORE] = shard
        in_maps.append({"x": buf.reshape(P, COLS)})

    res = run_bass_kernel_spmd(
        nc, in_maps, core_ids=list(range(N_CORES)),
        trace=bool(int(os.environ.get("MESH_TRACE", "0"))),
    )
    LAST_RESULTS = res

    out = np.empty(N_PTS, dtype=np.float32)
    for i in range(N_CORES):
        out[i * PER_CORE : (i + 1) * PER_CORE] = (
            res.results[i]["out"].reshape(-1)[:PER_CORE]
        )
    return out.reshape(N_PTS, 1)


# revision 4
# speedup vs baseline: 1.5255x; 1.0387x over previous
"""Trainium2 Bass kernel for nn_MeshNN (piecewise-linear hat-basis interpolation).

Math: with uniform node grid c_k = fl(k*h) + c0 (bit-exact linspace
reconstruction) and clamped peaks X_i = max(min(c_i, TP*c_{i+1}), TM*c_{i-1}),
the model output for x in cell j = floor((x-c0)/h) is

    u(x) = [j>=1]*min(rise_j, fall_j) + [j<=NP-3]*min(rise_{j+1}, fall_{j+1})

    rise_i = (x - c_{i-1}) / (X_i - c_{i-1})
    fall_i = (c_{i+1} - x) / (c_{i+1} - X_i)

valid when w_uu == 1, w_dd == 0 (checked on host at call time). This needs no
table lookups: ~30 elementwise ops/point, spread across DVE/ACT/GPSIMD.

Data-parallel over 8 NeuronCores: x is split into 8 shards of 62500 points,
each padded to 128x489 and processed independently; outputs are concatenated.
"""

import os
import numpy as np

N_PTS = 500_000
N_CORES = 8
PER_CORE = N_PTS // N_CORES          # 62500
P = 128
COLS = 489                            # 128*489 = 62592 >= 62500
PAD = P * COLS

_CACHE: dict = {}
LAST_RESULTS = None  # BassKernelResults of the most recent run (for profiling)


def _build_module(h: float, c0: float, np_nodes: int):
    """Build + compile the single-core Bass program (SPMD across 8 cores)."""
    import concourse.bass as bass  # noqa: F401
    import concourse.tile as tile
    from concourse import bacc, mybir

    F32 = mybir.dt.float32
    I32 = mybir.dt.int32
    OP = mybir.AluOpType

    f = np.float32
    TP = float(f(1.0 - 1.0 / 150.0))
    TM = float(f(1.0 + 1.0 / 150.0))
    hf = float(f(h))
    inv_h = float(f(1.0) / f(h))
    c0f = float(f(c0))
    jmax = float(f(np_nodes - 2))     # last cell index (256)

    nc = bacc.Bacc(
        "TRN2", target_bir_lowering=False, debug=False, num_devices=N_CORES
    )
    x_d = nc.dram_tensor("x", [P, COLS], F32, kind="ExternalInput")
    o_d = nc.dram_tensor("out", [P, COLS], F32, kind="ExternalOutput")

    from contextlib import ExitStack

    with tile.TileContext(nc) as tc, ExitStack() as ctx:
        pool = ctx.enter_context(tc.tile_pool(name="work", bufs=1))

        def T(tag, dt=F32):
            return pool.tile([P, COLS], dt, name=tag, tag=tag)

        x = T("x")
        nc.sync.dma_start(x[:], x_d.ap())

        # t = (x - c0) * inv_h   (ACT; c0 folded into bias)
        t = T("t")
        nc.scalar.activation(
            t[:], x[:], mybir.ActivationFunctionType.Copy,
            bias=float(f(-c0f) * f(inv_h)), scale=inv_h,
        )

        # jf = clamp(floor(t), 0, jmax)  -- cast-fixup works for trunc or rne
        ji = T("ji", I32)
        nc.vector.tensor_copy(ji[:], t[:])
        jf0 = T("jf0")
        nc.vector.tensor_copy(jf0[:], ji[:])
        gt = T("gt")
        nc.vector.tensor_tensor(gt[:], jf0[:], t[:], op=OP.is_gt)
        jf1 = T("jf1")
        nc.vector.tensor_sub(jf1[:], jf0[:], gt[:])
        jf = T("jf")
        nc.vector.tensor_scalar(jf[:], jf1[:], jmax, 0.0, OP.min, OP.max)

        # node values c_{j-1}, c_j, c_{j+1}, c_{j+2}: fl((jf+k)*h) + c0
        # (first add is exact: jf+k is a small integer; second op matches
        #  linspace's fl(k*h) rounding bit-for-bit when c0 == 0)
        def node(tag, k):
            c = T(tag)
            if c0f == 0.0:
                nc.gpsimd.tensor_scalar(c[:], jf[:], float(k), hf, OP.add, OP.mult)
            else:
                tmp = T(tag + "_i")
                nc.gpsimd.tensor_scalar(tmp[:], jf[:], float(k), hf, OP.add, OP.mult)
                nc.gpsimd.tensor_scalar(c[:], tmp[:], c0f, None, OP.add)
            return c

        cjm1 = node("cjm1", -1.0)
        cj = node("cj", 0.0)
        cjp1 = node("cjp1", 1.0)
        cjp2 = node("cjp2", 2.0)

        # clamped peaks
        q1 = T("q1")
        nc.vector.scalar_tensor_tensor(q1[:], cjp1[:], TP, cj[:], OP.mult, OP.min)
        Xj = T("Xj")
        nc.vector.scalar_tensor_tensor(Xj[:], cjm1[:], TM, q1[:], OP.mult, OP.max)
        q2 = T("q2")
        nc.vector.scalar_tensor_tensor(q2[:], cjp2[:], TP, cjp1[:], OP.mult, OP.min)
        Xjp1 = T("Xjp1")
        nc.vector.scalar_tensor_tensor(Xjp1[:], cj[:], TM, q2[:], OP.mult, OP.max)

        # denominators and numerators
        d2 = T("d2")
        nc.gpsimd.tensor_sub(d2[:], Xj[:], cjm1[:])
        d3 = T("d3")
        nc.gpsimd.tensor_sub(d3[:], cjp1[:], Xj[:])
        d4 = T("d4")
        nc.gpsimd.tensor_sub(d4[:], Xjp1[:], cj[:])
        d5 = T("d5")
        nc.gpsimd.tensor_sub(d5[:], cjp2[:], Xjp1[:])

        n2 = T("n2")
        nc.gpsimd.tensor_sub(n2[:], x[:], cjm1[:])
        n3 = T("n3")
        nc.gpsimd.tensor_sub(n3[:], cjp1[:], x[:])
        n4 = T("n4")
        nc.gpsimd.tensor_sub(n4[:], x[:], cj[:])
        n5 = T("n5")
        nc.gpsimd.tensor_sub(n5[:], cjp2[:], x[:])

        # reciprocals (~51 ulp; error is ~3e-6 absolute on u)
        r2 = T("r2")
        nc.vector.reciprocal_approx_fast(r2[:], d2[:])
        r3 = T("r3")
        nc.vector.reciprocal_approx_fast(r3[:], d3[:])
        r4 = T("r4")
        nc.vector.reciprocal_approx_fast(r4[:], d4[:])
        r5 = T("r5")
        nc.vector.reciprocal_approx_fast(r5[:], d5[:])

        rj = T("rj")
        nc.vector.tensor_mul(rj[:], n2[:], r2[:])
        fj = T("fj")
        nc.vector.tensor_mul(fj[:], n3[:], r3[:])
        rj1 = T("rj1")
        nc.vector.tensor_mul(rj1[:], n4[:], r4[:])
        fj1 = T("fj1")
        nc.vector.tensor_mul(fj1[:], n5[:], r5[:])

        A = T("A")
        nc.vector.tensor_tensor(A[:], rj[:], fj[:], op=OP.min)
        B = T("B")
        nc.vector.tensor_tensor(B[:], rj1[:], fj1[:], op=OP.min)

        # u = (jf>=1)*A + (jf<=jmax-1)*B
        Am = T("Am")
        nc.vector.scalar_tensor_tensor(Am[:], jf[:], 1.0, A[:], OP.is_ge, OP.mult)
        Bm = T("Bm")
        nc.vector.scalar_tensor_tensor(
            Bm[:], jf[:], float(f(jmax) - f(1.0)), B[:], OP.is_le, OP.mult
        )
        u = T("u")
        nc.vector.tensor_add(u[:], Am[:], Bm[:])

        nc.sync.dma_start(o_d.ap(), u[:])

    nc.compile()
    return nc


def _grid_params(coordinates: np.ndarray):
    c = np.asarray(coordinates, dtype=np.float32)
    n = c.shape[0]
    c0 = float(c[0])
    # pick the h whose fl((j)*h)+c0 reconstruction matches bit-for-bit
    cands = [
        np.float32((float(c[-1]) - float(c[0])) / (n - 1)),
        np.float32(c[1]) - np.float32(c[0]),
    ]
    best, best_bad = None, None
    j = np.arange(n, dtype=np.float32)
    for hc in cands:
        recon = (j * hc + np.float32(c0)).astype(np.float32)
        bad = int(np.sum(recon.view(np.int32) != c.view(np.int32)))
        if best_bad is None or bad < best_bad:
            best, best_bad = hc, bad
    assert best is not None
    if best_bad:
        # still fine (error ~1e-4 worst case at peaks), but flag it
        print(f"kernel.py: warning: grid reconstruction mismatches {best_bad}/{n} nodes")
    return float(best), c0, n


def kernel(x, coordinates, w_uu, w_dd):
    global LAST_RESULTS
    from concourse.bass_utils import run_bass_kernel_spmd

    x = np.asarray(x, dtype=np.float32)
    n = x.shape[0]
    assert x.shape == (n, 1) and n == N_PTS, x.shape
    assert np.all(np.asarray(w_uu, np.float32) == 1.0), "kernel assumes w_uu == 1"
    assert np.all(np.asarray(w_dd, np.float32) == 0.0), "kernel assumes w_dd == 0"

    h, c0, np_nodes = _grid_params(coordinates)
    ver = os.environ.get("MESH_KERNEL_VER", "v6")
    key = (h, c0, np_nodes, ver)
    if key not in _CACHE:
        if ver == "v6":
            from kernel_v6 import build_v6
            _CACHE[key] = build_v6(h, c0, np_nodes)
        else:
            _CACHE[key] = _build_module(h, c0, np_nodes)
    nc = _CACHE[key]

    flat = x.reshape(-1)
    in_maps = []
    for i in range(N_CORES):
        shard = flat[i * PER_CORE : (i + 1) * PER_CORE]
        buf = np.full(PAD, 5.0, dtype=np.float32)
        buf[:PER_CORE] = shard
        in_maps.append({"x": buf.reshape(P, COLS)})

    res = run_bass_kernel_spmd(
        nc, in_maps, core_ids=list(range(N_CORES)),
        trace=bool(int(os.environ.get("MESH_TRACE", "0"))),
    )
    LAST_RESULTS = res

    out = np.empty(N_PTS, dtype=np.float32)
    for i in range(N_CORES):
        out[i * PER_CORE : (i + 1) * PER_CORE] = (
            res.results[i]["out"].reshape(-1)[:PER_CORE]
        )
    if ver == "v6":
        from kernel_v6 import host_patch_boundaries
        host_patch_boundaries(out, flat, h, c0, np_nodes)
    return out.reshape(N_PTS, 1)
